# revision 1
# baseline (speedup 1.0000x reference)
"""Trainium2 Bass kernel for nn_CrossAttentionModel (cross-attention pooling).

Strategy
--------
Data-parallel over batch: core i handles batch item i (B=8, 8 cores, no
collectives).  The reference's huge [B,L2,L1,H]/[B,L2,L1,D] intermediates are
never materialized; instead the computation is refactored per pair (m,l):

    rh    = relu(H1[l] + H2[m] + tb1)          (H=1024)
    s     = rh @ W_a + b_a                     (W_a = tw2 @ aw1, folded)
    logit = relu(s) @ aw2 + ab2
    attn  = sigmoid(logit) * valid[m,l]
    P_h  += attn * rh ;  S += attn
    y     = (P_h . w_c + S * t_c) / (S + 1e-5) + cb   (w_c = tw2 @ cw)

which is algebraically identical to the reference (emb @ aw1 = rh @ (tw2@aw1)
+ tb2@aw1, and pooled.cw factors through tw2@cw).

On-chip layout keeps features on partitions and pairs on the free dim:
rh^T tiles [128, NP] feed the big s-matmul (fp32r, 1 cyc/row), the logit
matmul uses a column-replicated aw2 so sigmoid/masking/weighting run on all
128 partitions, and P_h/S are accumulated in exact fp32 on the vector engine.
H1/H2 are computed in true fp32 (value path is precision-sensitive).

Valid-pair compaction: rows l with mask1=0 and columns m with mask2=0
contribute nothing (attn=0), so the host compacts both index lists before
building the (static-shape) program; with Bernoulli(0.5) masks this cuts the
pair grid ~4x (capacities are maxed over the 8 cores, so ~2.3x in practice).
"""

import numpy as np

B, L1, L2, D, HH, V = 8, 64, 64, 768, 1024, 50257
PAD_ID = 50257
P = 128
DC = D // P    # 6 chunks of the 768 dims
HC = HH // P   # 8 chunks of the 1024 dims

_prog_cache = {}


def _build_program(N1, N2P, K, NBLK, ab2_f, cb_f, t_c_f, debug_stage=None):
    import concourse.bass as bass
    import concourse.bacc as bacc
    import concourse.mybir as mybir
    import concourse.tile as tile
    from concourse.masks import make_identity

    f32 = mybir.dt.float32
    bf16 = mybir.dt.bfloat16
    i32 = mybir.dt.int32
    Act = mybir.ActivationFunctionType
    Alu = mybir.AluOpType
    Axis = mybir.AxisListType

    NP = K * N1  # pairs per block
    NT = NBLK * NP

    nc = bacc.Bacc(
        "TRN2",
        target_bir_lowering=False,
        debug=False,
        enable_asserts=False,
        num_devices=8,
    )

    table = nc.dram_tensor("table", [V, D], f32, kind="ExternalInput").ap()
    idx1_d = nc.dram_tensor("idx1", [N1], i32, kind="ExternalInput").ap()
    idx2_d = nc.dram_tensor("idx2", [N2P], i32, kind="ExternalInput").ap()
    valid_d = nc.dram_tensor("valid", [NT], f32, kind="ExternalInput").ap()
    w1a_d = nc.dram_tensor("w1a", [D, HH], f32, kind="ExternalInput").ap()
    w1b_d = nc.dram_tensor("w1b", [D, HH], f32, kind="ExternalInput").ap()
    wa_d = nc.dram_tensor("W_a", [HH, D], bf16, kind="ExternalInput").ap()
    tb1_d = nc.dram_tensor("tb1v", [HH], f32, kind="ExternalInput").ap()
    ba_d = nc.dram_tensor("b_av", [D], f32, kind="ExternalInput").ap()
    aw2_d = nc.dram_tensor("aw2rep", [P, DC * P], bf16, kind="ExternalInput").ap()
    wc_d = nc.dram_tensor("w_cv", [HH], f32, kind="ExternalInput").ap()
    y_d = nc.dram_tensor("y", [1, 1], f32, kind="ExternalOutput").ap()
    dbg_d = None
    if debug_stage is not None:
        dbg_d = nc.dram_tensor("dbg", [P, 1024], f32, kind="ExternalOutput").ap()

    with tile.TileContext(nc, trace_sim=False) as tc:
        with (
            tc.tile_pool(name="const", bufs=1) as cpool,
            tc.tile_pool(name="wts", bufs=1) as wpool,
            tc.tile_pool(name="work", bufs=1) as work,
            tc.tile_pool(name="rh", bufs=1) as rhp,
            tc.tile_pool(name="ps", bufs=7, space="PSUM") as psp,
        ):
            ident = cpool.tile([P, P], f32)
            make_identity(nc, ident[:])
            ones_col = cpool.tile([P, 1], f32)
            nc.vector.memset(ones_col[:], 1.0)

            tb1c = cpool.tile([P, HC], f32)
            nc.sync.dma_start(tb1c[:], tb1_d.rearrange("(c p) -> p c", p=P))
            bac = cpool.tile([P, DC], f32)
            nc.sync.dma_start(bac[:], ba_d.rearrange("(c p) -> p c", p=P))
            wcc = cpool.tile([P, HC], f32)
            nc.sync.dma_start(wcc[:], wc_d.rearrange("(c p) -> p c", p=P))
            aw2rep = cpool.tile([P, DC * P], bf16)
            nc.sync.dma_start(aw2rep[:], aw2_d[:])
            valid_rep = cpool.tile([P, NT], f32)
            nc.sync.dma_start(
                valid_rep[:],
                valid_d.rearrange("(o t) -> o t", o=1).broadcast_to([P, NT]),
            )
            idx1_sb = cpool.tile([N1, 1], i32)
            nc.sync.dma_start(idx1_sb[:], idx1_d.rearrange("(p o) -> p o", o=1))
            idx2_sb = cpool.tile([N2P, 1], i32)
            nc.sync.dma_start(idx2_sb[:], idx2_d.rearrange("(p o) -> p o", o=1))

            w1a_sb = []
            w1b_sb = []
            wa_sb = []
            for dc in range(DC):
                t = wpool.tile([P, HH], f32, tag="w1a", bufs=DC, name=f"w1a_{dc}")
                nc.sync.dma_start(t[:], w1a_d[dc * P:(dc + 1) * P, :])
                w1a_sb.append(t)
            for dc in range(DC):
                t = wpool.tile([P, HH], f32, tag="w1b", bufs=DC, name=f"w1b_{dc}")
                nc.sync.dma_start(t[:], w1b_d[dc * P:(dc + 1) * P, :])
                w1b_sb.append(t)
            for hc in range(HC):
                t = wpool.tile([P, D], bf16, tag="wa", bufs=HC, name=f"wa_{hc}")
                nc.sync.dma_start(t[:], wa_d[hc * P:(hc + 1) * P, :])
                wa_sb.append(t)

            # ---- gather + transpose + first MLP (H1 = E1 @ w1a etc.) ----
            def build_HT(idx_sb, n, w_sb, label):
                E = work.tile([n, D], f32, tag=f"E_{label}", bufs=1, name=f"E_{label}")
                nc.gpsimd.indirect_dma_start(
                    out=E[:],
                    out_offset=None,
                    in_=table,
                    in_offset=bass.IndirectOffsetOnAxis(ap=idx_sb[:, :1], axis=0),
                )
                ET = []
                for dc in range(DC):
                    pt = psp.tile([P, n], f32, tag="ps", name=f"ptE_{label}_{dc}")
                    nc.tensor.transpose(pt[:], E[:, dc * P:(dc + 1) * P], ident[:n, :n])
                    t = work.tile([P, n], f32, tag=f"ET_{label}", bufs=DC,
                                  name=f"ET_{label}_{dc}")
                    nc.scalar.copy(t[:], pt[:])
                    ET.append(t)
                Hsb = work.tile([n, HH], f32, tag=f"H_{label}", bufs=1, name=f"H_{label}")
                for half in range(2):
                    ph = psp.tile([n, 512], f32, tag="ps", name=f"ph_{label}_{half}")
                    for dc in range(DC):
                        nc.tensor.matmul(
                            ph[:],
                            lhsT=ET[dc][:],
                            rhs=w_sb[dc][:, half * 512:(half + 1) * 512],
                            start=(dc == 0),
                            stop=(dc == DC - 1),
                        )
                    nc.scalar.copy(Hsb[:, half * 512:(half + 1) * 512], ph[:])
                HT = []
                for hc in range(HC):
                    pt2 = psp.tile([P, n], f32, tag="ps", name=f"ptH_{label}_{hc}")
                    nc.tensor.transpose(pt2[:], Hsb[:, hc * P:(hc + 1) * P], ident[:n, :n])
                    t = work.tile([P, n], f32, tag=f"HT_{label}", bufs=HC,
                                  name=f"HT_{label}_{hc}")
                    nc.scalar.copy(t[:], pt2[:])
                    HT.append(t)
                return HT

            stage = 99 if debug_stage is None else debug_stage
            if stage < 2:
                nc.sync.dma_start(dbg_d[:, 0:HC], tb1c[:])
                nc.sync.dma_start(dbg_d[:, 8:8 + 512], wa_sb[0][:, :512])
            if stage >= 2:
                H1T = build_HT(idx1_sb, N1, w1a_sb, "a")
                H2T = build_HT(idx2_sb, N2P, w1b_sb, "b")
            if stage == 2:
                nc.sync.dma_start(dbg_d[:, 0:N1], H1T[0][:])
                nc.sync.dma_start(dbg_d[:, 512:512 + N2P], H2T[0][:])

            if stage >= 6:
                S_parts = work.tile([P, NBLK], f32, tag="Sp", bufs=1)
            Ph_parts = []
            if stage >= 7:
                for hc in range(HC):
                    t = work.tile([P, NBLK], f32, tag="Php", bufs=HC, name=f"Php_{hc}")
                    Ph_parts.append(t)

            # ---- main pair-block loop ----
            for bi in range(NBLK if stage >= 3 else 0):
                rtfs = []
                rtbs = []
                for hc in range(HC):
                    rs = rhp.tile([P, NP], f32, tag="rs", bufs=HC + 1, name=f"rs_{bi}_{hc}")
                    nc.vector.tensor_tensor(
                        out=rs[:].rearrange("p (k l) -> p k l", k=K),
                        in0=H1T[hc][:].unsqueeze(1).broadcast_to([P, K, N1]),
                        in1=H2T[hc][:, bi * K:(bi + 1) * K].unsqueeze(2)
                            .broadcast_to([P, K, N1]),
                        op=Alu.add,
                    )
                    # fp32 relu copy (exact, for the P_h accumulation) on DVE
                    rtf = rhp.tile([P, NP], f32, tag="rtf", bufs=12, name=f"rtf_{bi}_{hc}")
                    nc.vector.tensor_scalar(
                        out=rtf[:], in0=rs[:],
                        scalar1=tb1c[:, hc:hc + 1], scalar2=0.0,
                        op0=Alu.add, op1=Alu.max,
                    )
                    # bf16 relu copy (matmul operand) on ACT
                    rtb = rhp.tile([P, NP], bf16, tag="rtb", bufs=12, name=f"rtb_{bi}_{hc}")
                    nc.scalar.activation(rtb[:], rs[:], Act.Relu, bias=tb1c[:, hc:hc + 1])
                    rtfs.append(rtf)
                    rtbs.append(rtb)

                if stage == 3:
                    if bi == 0:
                        nc.sync.dma_start(dbg_d[:, 0:NP], rtfs[0][:])
                    continue
                ats = []
                for dc in range(DC):
                    ps = psp.tile([P, NP], f32, tag="ps", name=f"ps_s_{bi}_{dc}")
                    for hc in range(HC):
                        nc.tensor.matmul(
                            ps[:],
                            lhsT=wa_sb[hc][:, dc * P:(dc + 1) * P],
                            rhs=rtbs[hc][:],
                            start=(hc == 0),
                            stop=(hc == HC - 1),
                        )
                    at = rhp.tile([P, NP], bf16, tag="at", bufs=DC + 1, name=f"at_{bi}_{dc}")
                    nc.scalar.activation(at[:], ps[:], Act.Relu, bias=bac[:, dc:dc + 1])
                    ats.append(at)

                if stage == 4:  # after s-matmul+at
                    if bi == 0:
                        atf = rhp.tile([P, NP], f32, tag="atf", bufs=1)
                        nc.vector.tensor_copy(atf[:], ats[0][:])
                        nc.sync.dma_start(dbg_d[:, 0:NP], atf[:])
                    continue
                pl = psp.tile([P, NP], f32, tag="ps", name=f"pl_{bi}")
                for dc in range(DC):
                    nc.tensor.matmul(
                        pl[:],
                        lhsT=aw2rep[:, dc * P:(dc + 1) * P],
                        rhs=ats[dc][:],
                        start=(dc == 0),
                        stop=(dc == DC - 1),
                    )
                attn = rhp.tile([P, NP], f32, tag="attn", bufs=2, name=f"attn_{bi}")
                nc.scalar.activation(attn[:], pl[:], Act.Sigmoid, bias=float(ab2_f))
                if stage == 5:  # after logit+sigmoid
                    if bi == 0:
                        nc.sync.dma_start(dbg_d[:, 0:NP], attn[:])
                    continue
                attnm = rhp.tile([P, NP], f32, tag="attnm", bufs=2, name=f"attnm_{bi}")
                nc.vector.tensor_tensor(
                    out=attnm[:], in0=attn[:],
                    in1=valid_rep[:, bi * NP:(bi + 1) * NP], op=Alu.mult,
                )
                nc.vector.tensor_reduce(
                    out=S_parts[:, bi:bi + 1], in_=attnm[:], axis=Axis.X, op=Alu.add,
                )
                if stage == 6:
                    if bi == 0:
                        nc.sync.dma_start(dbg_d[:, 0:NP], attnm[:])
                    continue
                for hc in range(HC):
                    scr = rhp.tile([P, NP], f32, tag="scr", bufs=3, name=f"scr_{bi}_{hc}")
                    nc.vector.tensor_tensor(
                        out=scr[:], in0=rtfs[hc][:], in1=attnm[:], op=Alu.mult,
                    )
                    nc.vector.tensor_reduce(
                        out=Ph_parts[hc][:, bi:bi + 1], in_=scr[:],
                        axis=Axis.X, op=Alu.add,
                    )

            # ---- final reduction: y = (P_h.w_c + S*t_c)/(S+1e-5) + cb ----
            if stage >= 7:
                Ph_all = work.tile([P, HC], f32, tag="Phall", bufs=1)
                for hc in range(HC):
                    nc.vector.tensor_reduce(
                        out=Ph_all[:, hc:hc + 1], in_=Ph_parts[hc][:],
                        axis=Axis.X, op=Alu.add,
                    )
                S_vec = work.tile([P, 1], f32, tag="Svec", bufs=1)
                nc.vector.tensor_reduce(out=S_vec[:], in_=S_parts[:], axis=Axis.X,
                                        op=Alu.add)
                yn_scr = work.tile([P, HC], f32, tag="ynscr", bufs=1)
                yn_vec = work.tile([P, 1], f32, tag="ynvec", bufs=1)
                nc.vector.tensor_tensor(out=yn_scr[:], in0=Ph_all[:], in1=wcc[:],
                                        op=Alu.mult)
                nc.vector.tensor_reduce(out=yn_vec[:], in_=yn_scr[:],
                                        axis=Axis.X, op=Alu.add)
                psy = psp.tile([1, 1], f32, tag="ps", name="psy")
                nc.tensor.matmul(psy[:], lhsT=yn_vec[:], rhs=ones_col[:],
                                 start=True, stop=True)

                den = work.tile([1, 1], f32, tag="den", bufs=1)
                nc.vector.tensor_scalar_add(den[:], S_vec[0:1, :], 1e-5)
                rden = work.tile([1, 1], f32, tag="rden", bufs=1)
                nc.vector.reciprocal(rden[:], den[:])
                num = work.tile([1, 1], f32, tag="num", bufs=1)
                nc.vector.scalar_tensor_tensor(
                    out=num[:], in0=S_vec[0:1, :], scalar=float(t_c_f), in1=psy[:],
                    op0=Alu.mult, op1=Alu.add,
                )
                y0 = work.tile([1, 1], f32, tag="y0", bufs=1)
                nc.vector.tensor_tensor(out=y0[:], in0=num[:], in1=rden[:], op=Alu.mult)
                y1 = work.tile([1, 1], f32, tag="y1", bufs=1)
                nc.vector.tensor_scalar_add(y1[:], y0[:], float(cb_f))
                nc.sync.dma_start(y_d[:], y1[:])

    nc.compile()
    return nc


def _prep(x1, x2, mask1, mask2, embed_table, tw1, tb1, tw2, tb2,
          aw1, ab1, aw2, ab2, cw, cb, compact=True):
    """Host-side sharding/index prep. Returns (program args, per-core in_maps)."""
    import ml_dtypes
    f32 = np.float32
    bf16 = ml_dtypes.bfloat16
    x1 = np.where(x1 == PAD_ID, 0, x1).astype(np.int32)
    x2 = np.where(x2 == PAD_ID, 0, x2).astype(np.int32)
    w1a = np.ascontiguousarray(tw1[:D]).astype(f32)
    w1b = np.ascontiguousarray(tw1[D:]).astype(f32)
    W_a = (tw2.astype(np.float64) @ aw1.astype(np.float64)).astype(f32)
    b_a = (tb2.astype(np.float64) @ aw1.astype(np.float64)
           + ab1.astype(np.float64)).astype(f32)
    w_c = (tw2.astype(np.float64) @ cw.astype(np.float64)).astype(f32).ravel()
    t_c = float(tb2.astype(np.float64) @ cw.astype(np.float64).ravel())

    if compact:
        l_lists = [np.nonzero(mask1[b])[0] for b in range(B)]
        m_lists = [np.nonzero(mask2[b])[0] for b in range(B)]
        N1 = max(4, max(len(l) for l in l_lists))
        N1 = (N1 + 3) & ~3
        N2 = max(1, max(len(m) for m in m_lists))
    else:
        l_lists = [np.arange(L1) for _ in range(B)]
        m_lists = [np.arange(L2) for _ in range(B)]
        N1, N2 = L1, L2
    K = max(1, min(16, 512 // N1))
    NBLK = -(-N2 // K)
    N2P = NBLK * K
    NP = K * N1
    NT = NBLK * NP

    table_f32 = np.ascontiguousarray(embed_table, dtype=f32)
    # aw2 replicated across matmul output columns: aw2rep[k, c*128+m] = aw2[c*128+k]
    aw2rep_host = np.ascontiguousarray(np.broadcast_to(
        aw2.astype(f32).ravel().reshape(DC, P).T[:, :, None], (P, DC, P)
    ).reshape(P, DC * P)).astype(bf16)
    in_maps = []
    for b in range(B):
        ll, ml = l_lists[b], m_lists[b]
        idx1 = np.zeros(N1, np.int32)
        idx1[:len(ll)] = x1[b][ll]
        idx2 = np.zeros(N2P, np.int32)
        idx2[:len(ml)] = x2[b][ml]
        valid = np.zeros((N2P, N1), f32)
        if len(ll) and len(ml):
            vm = (mask1[b][ll][None, :] != 0) & (mask2[b][ml][:, None] != 0) \
                 & (x1[b][ll][None, :] != x2[b][ml][:, None])
            valid[:len(ml), :len(ll)] = vm.astype(f32)
        in_maps.append({
            "table": table_f32,
            "idx1": idx1,
            "idx2": idx2,
            "valid": valid.ravel(),
            "w1a": w1a,
            "w1b": w1b,
            "W_a": W_a.astype(bf16),
            "tb1v": tb1.astype(f32),
            "b_av": b_a,
            "aw2rep": aw2rep_host,
            "w_cv": w_c,
        })
    ab2_f = float(np.asarray(ab2).ravel()[0])
    cb_f = float(np.asarray(cb).ravel()[0])
    return (N1, N2P, K, NBLK, ab2_f, cb_f, t_c), in_maps


def kernel(x1, x2, mask1, mask2, embed_table, tw1, tb1, tw2, tb2,
           aw1, ab1, aw2, ab2, cw, cb):
    from concourse import bass_utils

    (N1, N2P, K, NBLK, ab2_f, cb_f, t_c), in_maps = _prep(
        x1, x2, mask1, mask2, embed_table, tw1, tb1, tw2, tb2,
        aw1, ab1, aw2, ab2, cw, cb)

    key = (N1, N2P, K, NBLK, ab2_f, cb_f, t_c)
    if key not in _prog_cache:
        _prog_cache[key] = _build_program(*key)
    nc = _prog_cache[key]

    res = bass_utils.run_bass_kernel_spmd(nc, in_maps, core_ids=list(range(8)))
    y = np.stack([res.results[i]["y"].reshape(()) for i in range(B)])
    return y.reshape(B, 1).astype(np.float32)



# revision 9
# speedup vs baseline: 1.4288x; 1.4288x over previous
"""Trainium2 Bass kernel for nn_CrossAttentionModel (cross-attention pooling).

Strategy (v2)
-------------
Data-parallel over batch: core i handles batch item i (B=8, 8 cores, no
collectives).  Per pair (m,l) the computation is refactored as

    rh    = relu(H1[l] + H2[m] + tb1)            (H=1024)
    s     = rh @ W_a                             (W_a = tw2 @ aw1, folded)
    logit = relu(s + b_a) @ aw2 + ab2
    attn  = sigmoid(logit) * valid[m,l]
    y_num += attn * (rh . w_c);  S += attn       (w_c = tw2 @ cw, folded)
    y     = (y_num + S * t_c) / (S + 1e-5) + cb

Engine allocation per pair-block (NP pairs, feature chunks on partitions):
  * DVE    : rs = H1+H2b broadcast-add (fp32, 8 chunks), 3 of the 8 fp8
             quant chunks (fused mult+relu tensor_scalar), fused
             tensor_tensor_reduce for S (attn*valid -> sum) and for the
             value dot (g * attn -> sum), 3 per-chunk pool reduces.
  * ACT    : rhb = bf16(alpha*relu(rs)) (pool operand), 5 fp8 quant chunks,
             at = fp8 relu of the s-matmul PSUM (scale+bias fused),
             sigmoid.
  * TensorE: s-matmul in fp8 DoubleRow (2 rows/cycle), logit matmul in
             fp8 DoubleRow against a column-replicated aw2, value dot
             g = rhb @ w_c as 1-column bf16 matmuls, fp32r front-end
             (embedding MLP) at 1 cycle/row.
The precision split was validated numerically: the s/logit path tolerates
fp8 e4m3 end-to-end (errors cancel through the sigmoid), while the pooled
value path needs bf16 rh and fp32/bf16 w_c (fp8 there gives ~5e-2 error).

Valid-pair compaction (host): rows/cols with mask=0 are dropped before the
(static-shape) program is built; K*N1 <= 512 pair blocks, NBLK blocks.
"""

import numpy as np

B, L1, L2, D, HH, V = 8, 64, 64, 768, 1024, 50257
PAD_ID = 50257
P = 128
DC = D // P    # 6 chunks of the 768 dims
HC = HH // P   # 8 chunks of the 1024 dims

NG = HC                      # all feature chunks pooled via PE g-matmul
DVE_RTB = (4, 5, 6, 7)       # fp8 quant chunks produced on DVE
ACT_RTB = (0, 1, 2, 3)       # fp8 quant chunks produced on ACT

_prog_cache = {}


def _build_program(N1, N2P, K, NBLK, ab2_f, cb_f, t_c_f):
    import concourse.bass as bass
    import concourse.bacc as bacc
    import concourse.mybir as mybir
    import concourse.tile as tile
    from concourse.masks import make_identity

    f32 = mybir.dt.float32
    f32r = mybir.dt.float32r
    bf16 = mybir.dt.bfloat16
    f8 = mybir.dt.float8e4
    i32 = mybir.dt.int32
    Act = mybir.ActivationFunctionType
    Alu = mybir.AluOpType
    Axis = mybir.AxisListType
    DR = mybir.MatmulPerfMode.DoubleRow

    NP = K * N1
    NT = NBLK * NP

    nc = bacc.Bacc(
        "TRN2",
        target_bir_lowering=False,
        debug=False,
        enable_asserts=False,
        num_devices=8,
    )

    table = nc.dram_tensor("table", [V, D], f32, kind="ExternalInput").ap()
    idx1_d = nc.dram_tensor("idx1", [N1], i32, kind="ExternalInput").ap()
    idx2_d = nc.dram_tensor("idx2", [N2P], i32, kind="ExternalInput").ap()
    mrow_d = nc.dram_tensor("mrow", [NT], bf16, kind="ExternalInput").ap()
    w1a_d = nc.dram_tensor("w1a", [D, HH], f32r, kind="ExternalInput").ap()
    w1b_d = nc.dram_tensor("w1b", [D, HH], f32r, kind="ExternalInput").ap()
    wa4_d = nc.dram_tensor("wa4", [4 * P, 2 * D], f8, kind="ExternalInput").ap()
    aw2r_d = nc.dram_tensor("aw2r", [3 * P, 2 * P], f8, kind="ExternalInput").ap()
    wgc_d = nc.dram_tensor("wgc", [P, NG], f32r, kind="ExternalInput").ap()
    bat_d = nc.dram_tensor("bat", [P, DC], f32, kind="ExternalInput").ap()
    sat_d = nc.dram_tensor("sat", [P, 1], f32, kind="ExternalInput").ap()
    srh_d = nc.dram_tensor("srh", [P, 1], f32, kind="ExternalInput").ap()
    ssig_d = nc.dram_tensor("ssig", [P, 1], f32, kind="ExternalInput").ap()
    tb1_d = nc.dram_tensor("tb1c", [P, HC], f32, kind="ExternalInput").ap()
    y_d = nc.dram_tensor("y", [1, 1], f32, kind="ExternalOutput").ap()

    with tile.TileContext(nc, trace_sim=False) as tc:
        with (
            tc.tile_pool(name="const", bufs=1) as cpool,
            tc.tile_pool(name="wts", bufs=1) as wpool,
            tc.tile_pool(name="work", bufs=1) as work,
            tc.tile_pool(name="blk", bufs=1) as blk,
            tc.tile_pool(name="ps", bufs=8, space="PSUM") as psp,
        ):
            ident = cpool.tile([P, P], f32)
            make_identity(nc, ident[:])
            ones_col = cpool.tile([P, 1], f32)
            nc.vector.memset(ones_col[:], 1.0)

            tb1c = cpool.tile([P, HC], f32)
            nc.sync.dma_start(tb1c[:], tb1_d[:])
            bat = cpool.tile([P, DC], f32)
            nc.sync.dma_start(bat[:], bat_d[:])
            sat = cpool.tile([P, 1], f32)
            nc.sync.dma_start(sat[:], sat_d[:])
            srh = cpool.tile([P, 1], f32)
            nc.sync.dma_start(srh[:], srh_d[:])
            ssig = cpool.tile([P, 1], f32)
            nc.sync.dma_start(ssig[:], ssig_d[:])
            wgc = cpool.tile([P, NG], f32r)
            nc.sync.dma_start(wgc[:], wgc_d[:])
            mrow_sb = cpool.tile([1, NT], bf16)
            nc.sync.dma_start(mrow_sb[:], mrow_d.rearrange("(o t) -> o t", o=1))
            ones_row = cpool.tile([1, P], bf16)
            nc.vector.memset(ones_row[:], 1.0)
            idx1_sb = cpool.tile([N1, 1], i32)
            nc.sync.dma_start(idx1_sb[:], idx1_d.rearrange("(p o) -> p o", o=1))
            idx2_sb = cpool.tile([N2P, 1], i32)
            nc.sync.dma_start(idx2_sb[:], idx2_d.rearrange("(p o) -> p o", o=1))

            w1a_sb = []
            w1b_sb = []
            for dc in range(DC):
                t = wpool.tile([P, HH], f32r, tag="w1a", bufs=DC, name=f"w1a_{dc}")
                nc.sync.dma_start(t[:], w1a_d[dc * P:(dc + 1) * P, :])
                w1a_sb.append(t)
            for dc in range(DC):
                t = wpool.tile([P, HH], f32r, tag="w1b", bufs=DC, name=f"w1b_{dc}")
                nc.sync.dma_start(t[:], w1b_d[dc * P:(dc + 1) * P, :])
                w1b_sb.append(t)
            wa4 = []
            for j in range(4):
                t = wpool.tile([P, 2 * D], f8, tag="wa4", bufs=4, name=f"wa4_{j}")
                nc.sync.dma_start(t[:], wa4_d[j * P:(j + 1) * P, :])
                wa4.append(t)
            aw2r = []
            for j in range(3):
                t = wpool.tile([P, 2 * P], f8, tag="aw2r", bufs=3, name=f"aw2r_{j}")
                nc.sync.dma_start(t[:], aw2r_d[j * P:(j + 1) * P, :])
                aw2r.append(t)

            # ---- front end: gather + transpose + first MLP (fp32r) ----
            def build_HT(idx_sb, n, w_sb, label, add_tb1):
                E = work.tile([n, D], f32, tag=f"E_{label}", bufs=1, name=f"E_{label}")
                nc.gpsimd.indirect_dma_start(
                    out=E[:],
                    out_offset=None,
                    in_=table,
                    in_offset=bass.IndirectOffsetOnAxis(ap=idx_sb[:, :1], axis=0),
                )
                ET = []
                for dc in range(DC):
                    pt = psp.tile([P, n], f32, tag="ps", name=f"ptE_{label}_{dc}")
                    nc.tensor.transpose(pt[:], E[:, dc * P:(dc + 1) * P], ident[:n, :n])
                    t = work.tile([P, n], f32r, tag=f"ET_{label}", bufs=DC,
                                  name=f"ET_{label}_{dc}")
                    nc.scalar.copy(t[:], pt[:])
                    ET.append(t)
                Hsb = work.tile([n, HH], f32, tag=f"H_{label}", bufs=1,
                                name=f"H_{label}")
                for half in range(2):
                    ph = psp.tile([n, 512], f32, tag="ps", name=f"ph_{label}_{half}")
                    for dc in range(DC):
                        nc.tensor.matmul(
                            ph[:],
                            lhsT=ET[dc][:],
                            rhs=w_sb[dc][:, half * 512:(half + 1) * 512],
                            start=(dc == 0),
                            stop=(dc == DC - 1),
                        )
                    nc.scalar.copy(Hsb[:, half * 512:(half + 1) * 512], ph[:])
                HT = work.tile([P, HC * n], f32, tag=f"HT_{label}", bufs=1,
                               name=f"HT_{label}")
                for hc in range(HC):
                    pt2 = psp.tile([P, n], f32, tag="ps", name=f"ptH_{label}_{hc}")
                    nc.tensor.transpose(pt2[:], Hsb[:, hc * P:(hc + 1) * P],
                                        ident[:n, :n])
                    if add_tb1:
                        nc.vector.tensor_scalar_add(
                            HT[:, hc * n:(hc + 1) * n], pt2[:],
                            tb1c[:, hc:hc + 1],
                        )
                    else:
                        nc.scalar.copy(HT[:, hc * n:(hc + 1) * n], pt2[:])
                return HT

            H1T = build_HT(idx1_sb, N1, w1a_sb, "a", add_tb1=False)
            H2bT = build_HT(idx2_sb, N2P, w1b_sb, "b", add_tb1=True)

            # ---- per-batch accumulators ----
            S_parts = cpool.tile([P, NBLK], f32, name="S_parts")
            g_parts = cpool.tile([1, NBLK], f32, name="g_parts")

            rs_t = [[None] * HC for _ in range(NBLK)]
            rhb_t = [[None] * HC for _ in range(NBLK)]
            rtb_t = [[None] * 4 for _ in range(NBLK)]
            at_t = [[None] * 3 for _ in range(NBLK)]
            attn_t = [None] * NBLK
            gps_t = [None] * NBLK

            def emit_rs(bi):
                for hc in range(HC):
                    rs = blk.tile([P, NP], f32, tag="rs", bufs=12,
                                  name=f"rs_{bi}_{hc}")
                    nc.vector.tensor_tensor(
                        out=rs[:].rearrange("p (k l) -> p k l", k=K),
                        in0=H1T[:, hc * N1:(hc + 1) * N1].unsqueeze(1)
                            .broadcast_to([P, K, N1]),
                        in1=H2bT[:, hc * N2P + bi * K:hc * N2P + (bi + 1) * K]
                            .unsqueeze(2).broadcast_to([P, K, N1]),
                        op=Alu.add,
                    )
                    rs_t[bi][hc] = rs
                    if hc in DVE_RTB:
                        _emit_rtb_one(bi, hc, on_dve=True)

            def _rtb_slice(bi, hc):
                j, jj = hc // 2, hc % 2
                if rtb_t[bi][j] is None:
                    rtb_t[bi][j] = blk.tile([P, 2 * NP], f8, tag="rtb", bufs=8,
                                            name=f"rtb_{bi}_{j}")
                return rtb_t[bi][j][:, jj * NP:(jj + 1) * NP]

            def _emit_rtb_one(bi, hc, on_dve):
                out = _rtb_slice(bi, hc)
                if on_dve:
                    nc.vector.tensor_scalar(
                        out=out, in0=rs_t[bi][hc][:],
                        scalar1=srh[:, 0:1], scalar2=0.0,
                        op0=Alu.mult, op1=Alu.max,
                    )
                else:
                    nc.scalar.activation(out, rs_t[bi][hc][:], Act.Relu,
                                         scale=srh[:, 0:1])

            def emit_act_side(bi):
                for hc in ACT_RTB:
                    _emit_rtb_one(bi, hc, on_dve=False)
                for hc in range(HC):
                    rhb = blk.tile([P, NP], f32r, tag="rhb", bufs=20,
                                   name=f"rhb_{bi}_{hc}")
                    nc.scalar.activation(rhb[:], rs_t[bi][hc][:], Act.Relu,
                                         scale=srh[:, 0:1])
                    rhb_t[bi][hc] = rhb

            def emit_smm(bi):
                for dc in range(DC):
                    ps = psp.tile([P, NP], f32, tag="ps", name=f"ps_{bi}_{dc}")
                    for j in range(4):
                        nc.tensor.matmul(
                            ps[:],
                            lhsT=wa4[j][:].rearrange("p (t d) -> p t d", t=2)
                                [:, :, dc * P:(dc + 1) * P],
                            rhs=rtb_t[bi][j][:].rearrange("p (t n) -> p t n", t=2),
                            start=(j == 0),
                            stop=(j == 3),
                            perf_mode=DR,
                        )
                    dcp, dcj = dc // 2, dc % 2
                    if at_t[bi][dcp] is None:
                        at_t[bi][dcp] = blk.tile([P, 2 * NP], f8, tag="at", bufs=6,
                                                 name=f"at_{bi}_{dcp}")
                    nc.scalar.activation(
                        at_t[bi][dcp][:, dcj * NP:(dcj + 1) * NP], ps[:],
                        Act.Relu, bias=bat[:, dc:dc + 1], scale=sat[:, 0:1],
                    )

            def emit_logit_g_sig(bi):
                pl = psp.tile([P, NP], f32, tag="ps", name=f"pl_{bi}")
                for j in range(3):
                    nc.tensor.matmul(
                        pl[:],
                        lhsT=aw2r[j][:].rearrange("p (t d) -> p t d", t=2),
                        rhs=at_t[bi][j][:].rearrange("p (t n) -> p t n", t=2),
                        start=(j == 0),
                        stop=False,
                        perf_mode=DR,
                    )
                # additive -50/ssig mask for invalid pairs (attn -> 0)
                nc.tensor.matmul(
                    pl[:],
                    lhsT=ones_row[:],
                    rhs=mrow_sb[:, bi * NP:(bi + 1) * NP],
                    start=False,
                    stop=True,
                )
                gps = psp.tile([1, NP], f32, tag="ps", name=f"gps_{bi}")
                for i in range(NG):
                    nc.tensor.matmul(
                        gps[:],
                        lhsT=wgc[:, i:i + 1],
                        rhs=rhb_t[bi][i][:],
                        start=(i == 0),
                        stop=(i == NG - 1),
                    )
                gps_t[bi] = gps
                attn = blk.tile([P, NP], f32, tag="attn", bufs=2,
                                name=f"attn_{bi}")
                nc.scalar.activation(attn[:], pl[:], Act.Sigmoid,
                                     bias=float(ab2_f), scale=ssig[:, 0:1],
                                     accum_out=S_parts[:, bi:bi + 1])
                attn_t[bi] = attn

            def emit_tail(bi):
                gscr = blk.tile([1, NP], f32, tag="gscr", bufs=2,
                                name=f"gscr_{bi}")
                nc.vector.tensor_tensor(
                    out=gscr[:], in0=gps_t[bi][:], in1=attn_t[bi][0:1, :],
                    op=Alu.mult,
                )
                nc.vector.tensor_reduce(
                    out=g_parts[:, bi:bi + 1], in_=gscr[:],
                    axis=Axis.X, op=Alu.add,
                )

            for bi in range(NBLK):
                emit_rs(bi)
                if bi > 0:
                    emit_tail(bi - 1)
                emit_act_side(bi)
                emit_smm(bi)
                emit_logit_g_sig(bi)
            emit_tail(NBLK - 1)

            # ---- epilogue ----
            S_tot = work.tile([1, 1], f32, tag="Stot", bufs=1)
            nc.vector.tensor_reduce(out=S_tot[:], in_=S_parts[0:1, :],
                                    axis=Axis.X, op=Alu.add)
            g_tot = work.tile([1, 1], f32, tag="gtot", bufs=1)
            nc.vector.tensor_reduce(out=g_tot[:], in_=g_parts[0:1, :],
                                    axis=Axis.X, op=Alu.add)
            num2 = work.tile([1, 1], f32, tag="num2", bufs=1)
            nc.vector.scalar_tensor_tensor(
                out=num2[:], in0=S_tot[:], scalar=float(t_c_f), in1=g_tot[:],
                op0=Alu.mult, op1=Alu.add,
            )
            den = work.tile([1, 1], f32, tag="den", bufs=1)
            nc.vector.tensor_scalar_add(den[:], S_tot[:], 1e-5)
            rden = work.tile([1, 1], f32, tag="rden", bufs=1)
            nc.vector.reciprocal(rden[:], den[:])
            y0 = work.tile([1, 1], f32, tag="y0", bufs=1)
            nc.vector.tensor_tensor(out=y0[:], in0=num2[:], in1=rden[:],
                                    op=Alu.mult)
            y1 = work.tile([1, 1], f32, tag="y1", bufs=1)
            nc.vector.tensor_scalar_add(y1[:], y0[:], float(cb_f))
            nc.sync.dma_start(y_d[:], y1[:])

    nc.compile()
    return nc


def _prep(x1, x2, mask1, mask2, embed_table, tw1, tb1, tw2, tb2,
          aw1, ab1, aw2, ab2, cw, cb):
    """Host-side sharding/scale prep. Returns (program args, per-core in_maps)."""
    import ml_dtypes
    f32 = np.float32
    bf16 = ml_dtypes.bfloat16
    fp8 = ml_dtypes.float8_e4m3
    x1 = np.where(x1 == PAD_ID, 0, x1).astype(np.int32)
    x2 = np.where(x2 == PAD_ID, 0, x2).astype(np.int32)
    w1a = np.ascontiguousarray(tw1[:D]).astype(f32)
    w1b = np.ascontiguousarray(tw1[D:]).astype(f32)
    W_a = (tw2.astype(np.float64) @ aw1.astype(np.float64)).astype(f32)
    b_a = (tb2.astype(np.float64) @ aw1.astype(np.float64)
           + ab1.astype(np.float64)).astype(f32)
    w_c = (tw2.astype(np.float64) @ cw.astype(np.float64)).astype(f32).ravel()
    t_c = float(tb2.astype(np.float64) @ cw.astype(np.float64).ravel())

    l_lists = [np.nonzero(mask1[b])[0] for b in range(B)]
    m_lists = [np.nonzero(mask2[b])[0] for b in range(B)]
    N1 = max(4, max(len(l) for l in l_lists))
    N1 = (N1 + 3) & ~3
    N2 = max(1, max(len(m) for m in m_lists))
    Kmax = max(1, min(16, 512 // N1))
    NBLK = -(-N2 // Kmax)
    K = -(-N2 // NBLK)
    N2P = NBLK * K
    NP = K * N1
    NT = NBLK * NP

    table_f32 = np.ascontiguousarray(embed_table, dtype=f32)

    idx1s, idx2s, valids = [], [], []
    amax, smax = 1e-6, 1e-6
    for b in range(B):
        ll, ml = l_lists[b], m_lists[b]
        idx1 = np.zeros(N1, np.int32)
        idx1[:len(ll)] = x1[b][ll]
        idx2 = np.zeros(N2P, np.int32)
        idx2[:len(ml)] = x2[b][ml]
        valid = np.zeros((N2P, N1), f32)
        if len(ll) and len(ml):
            vm = (mask1[b][ll][None, :] != 0) & (mask2[b][ml][:, None] != 0) \
                 & (x1[b][ll][None, :] != x2[b][ml][:, None])
            valid[:len(ml), :len(ll)] = vm.astype(f32)
        idx1s.append(idx1)
        idx2s.append(idx2)
        valids.append(valid)
        # scale bounds from the padded index lists (exact upper bounds)
        H1 = table_f32[idx1] @ w1a
        H2b = table_f32[idx2] @ w1b + tb1.astype(f32)
        u = np.maximum(H1.max(0) + H2b.max(0), 0)
        amax = max(amax, float(u.max()))
        smax = max(smax, float((u @ np.abs(W_a)).max() + np.abs(b_a).max()))

    alpha = 224.0 / amax
    beta = 240.0 / max(float(np.abs(W_a).max()), 1e-30)
    gamma = 224.0 / smax
    delta = 240.0 / max(float(np.abs(aw2).max()), 1e-30)

    def to_fp8(x):
        return np.clip(x, -240, 240).astype(fp8)

    ssig_val = 1.0 / (gamma * delta)
    # wa4[j][p, t*D+d] = fp8(beta * W_a[(2j+t)*128+p, d])
    Wa_s = (W_a * beta).reshape(4, 2, P, D).transpose(0, 2, 1, 3).reshape(4 * P, 2 * D)
    wa4_host = to_fp8(Wa_s)
    # aw2r[j][p, t*128+i] = fp8(delta * aw2[(2j+t)*128+p]) for all i
    a2 = (aw2.astype(f32).ravel() * delta).reshape(3, 2, P)
    aw2r_host = to_fp8(np.broadcast_to(
        a2.transpose(0, 2, 1)[:, :, :, None], (3, P, 2, P)
    ).reshape(3 * P, 2 * P))
    # value-dot weights; rhb carries alpha*rh so fold 1/alpha here
    wgc_host = np.ascontiguousarray(
        (w_c.reshape(NG, P).T / alpha).astype(f32))
    bat_host = np.ascontiguousarray((b_a * gamma).reshape(DC, P).T).astype(f32)
    sat_host = np.full((P, 1), gamma / (alpha * beta), f32)
    srh_host = np.full((P, 1), alpha, f32)
    ssig_host = np.full((P, 1), 1.0 / (gamma * delta), f32)
    tb1_host = np.ascontiguousarray(tb1.astype(f32).reshape(HC, P).T)

    in_maps = []
    for b in range(B):
        in_maps.append({
            "table": table_f32,
            "idx1": idx1s[b],
            "idx2": idx2s[b],
            "mrow": ((1.0 - valids[b].ravel()) * (-50.0 / ssig_val)).astype(bf16),
            "w1a": w1a,
            "w1b": w1b,
            "wa4": wa4_host,
            "aw2r": aw2r_host,
            "wgc": wgc_host,
            "bat": bat_host,
            "sat": sat_host,
            "srh": srh_host,
            "ssig": ssig_host,
            "tb1c": tb1_host,
        })
    ab2_f = float(np.asarray(ab2).ravel()[0])
    cb_f = float(np.asarray(cb).ravel()[0])
    return (N1, N2P, K, NBLK, ab2_f, cb_f, t_c), in_maps


def kernel(x1, x2, mask1, mask2, embed_table, tw1, tb1, tw2, tb2,
           aw1, ab1, aw2, ab2, cw, cb):
    from concourse import bass_utils

    args, in_maps = _prep(
        x1, x2, mask1, mask2, embed_table, tw1, tb1, tw2, tb2,
        aw1, ab1, aw2, ab2, cw, cb)

    if args not in _prog_cache:
        _prog_cache[args] = _build_program(*args)
    nc = _prog_cache[args]

    res = bass_utils.run_bass_kernel_spmd(nc, in_maps, core_ids=list(range(8)))
    y = np.stack([res.results[i]["y"].reshape(()) for i in range(B)])
    return y.reshape(B, 1).astype(np.float32)
